# revision 7
# baseline (speedup 1.0000x reference)
"""Multi-head attention (B=2, S=2048, D=1024, H=16, d_k=64) on 8 TRN2 NeuronCores.

Sharding: data + tensor parallel over [B, H]. Core c handles batch c//4 and
head group c%4 (4 heads, 256 of the 1024 projection dims). Wq/Wk/Wv are split
column-wise, Wo row-wise; each core emits a partial [S, D] output which the
host sums per batch (+ bo).

Per-core dataflow (all matmul operands float32r -> full PE rate, ~1e-4 err):
  A) qT/kT = (Wq|Wk)^T X^T + b   [256, 2048] transposed layout, PE + DVE bias
     v = X Wv + bv               [2048, 4*65] natural layout, ones-col per head
  B) per head pair, per 512-col sq quarter, per 128-row sk tile:
       ST[sk, sq] = k q^T        (2 heads packed in PE row halves)
       E = exp(ST/8)             (one ACT op per [128, 1024] tile)
       OT[65, sq] += [v|1]^T E   (row 64 accumulates the softmax denominator)
     then OT[0:64] *= 1/OT[64] via DVE recip + gpsimd partition-broadcast
  C) yp[t, :] = sum_c otn[:, c, t]^T wo[c]  (contract 256 head dims), DMA out.
"""

import numpy as np

import concourse.bacc as bacc
import concourse.mybir as mybir
import concourse.tile as tile
from concourse.bass_utils import run_bass_kernel_spmd

dt = mybir.dt

S = 2048
D = 1024
DH = 256  # head dims per core (4 heads x 64)
DK = 64
P = 128
NK = D // P  # 8 contraction chunks for projections
NM = DH // P  # 2 row groups of qT/kT
NST = S // P  # 16 sk tiles
NQ4 = S // 512  # 4 sq quarters
NCORES = 8
VW = 65  # v columns per head incl. ones column

F32R = dt.float32r
F32 = dt.float32


def _build_program():
    nc = bacc.Bacc("TRN2", target_bir_lowering=False, debug=False,
                   num_devices=NCORES)

    xqT = nc.dram_tensor("xqT", [D, S], F32R, kind="ExternalInput").ap()
    xkT = nc.dram_tensor("xkT", [D, S], F32R, kind="ExternalInput").ap()
    xvT = nc.dram_tensor("xvT", [D, S], F32R, kind="ExternalInput").ap()
    wq = nc.dram_tensor("wq", [D, DH], F32R, kind="ExternalInput").ap()
    wk = nc.dram_tensor("wk", [D, DH], F32R, kind="ExternalInput").ap()
    wv = nc.dram_tensor("wv", [D, DH], F32R, kind="ExternalInput").ap()
    wo = nc.dram_tensor("wo", [DH, D], F32R, kind="ExternalInput").ap()
    bq = nc.dram_tensor("bq", [DH, 1], F32, kind="ExternalInput").ap()
    bk = nc.dram_tensor("bk", [DH, 1], F32, kind="ExternalInput").ap()
    bv = nc.dram_tensor("bv", [1, DH], F32R, kind="ExternalInput").ap()
    onesd = nc.dram_tensor("onesd", [1, P], F32R, kind="ExternalInput").ap()
    vones = nc.dram_tensor("vones", [P, NST * 4], F32R, kind="ExternalInput").ap()
    yp = nc.dram_tensor("yp", [S, D], F32, kind="ExternalOutput").ap()

    with tile.TileContext(nc) as tc:
        with tc.tile_pool(name="persist", bufs=1) as pp_sb, \
             tc.tile_pool(name="xq_pool", bufs=6) as xq_pool, \
             tc.tile_pool(name="xv_pool", bufs=6) as xv_pool, \
             tc.tile_pool(name="e_pool", bufs=3) as e_pool, \
             tc.tile_pool(name="nrm_pool", bufs=4) as nrm_pool, \
             tc.tile_pool(name="y_pool", bufs=3) as y_pool:

            # ---- persistent SBUF ----
            wq_sb = pp_sb.tile([P, NK, DH], F32R, tag="wq_sb")
            wk_sb = pp_sb.tile([P, NK, DH], F32R, tag="wk_sb")
            wv_sb = pp_sb.tile([P, NK, DH], F32R, tag="wv_sb")
            wo_sb = pp_sb.tile([P, NM, D], F32R, tag="wo_sb")
            bq_sb = pp_sb.tile([P, NM], F32, tag="bq_sb")
            bk_sb = pp_sb.tile([P, NM], F32, tag="bk_sb")
            bv_sb = pp_sb.tile([1, DH], F32R, tag="bv_sb")
            ones_sb = pp_sb.tile([1, P], F32R, tag="ones_sb")
            qT_sb = pp_sb.tile([P, NM, S], F32R, tag="qT_sb")
            kT_sb = pp_sb.tile([P, NM, S], F32R, tag="kT_sb")
            v_sb = pp_sb.tile([P, NST, 4 * VW], F32R, tag="v_sb")
            otn_sb = pp_sb.tile([P, NM, S], F32R, tag="otn_sb")

            nc.sync.dma_start(out=wq_sb, in_=wq.rearrange("(k p) n -> p k n", p=P))
            nc.sync.dma_start(out=wk_sb, in_=wk.rearrange("(k p) n -> p k n", p=P))
            nc.sync.dma_start(out=wv_sb, in_=wv.rearrange("(k p) n -> p k n", p=P))
            nc.sync.dma_start(out=wo_sb, in_=wo.rearrange("(c p) n -> p c n", p=P))
            nc.sync.dma_start(out=bq_sb, in_=bq.rearrange("(m p) o -> p (m o)", p=P))
            nc.sync.dma_start(out=bk_sb, in_=bk.rearrange("(m p) o -> p (m o)", p=P))
            nc.sync.dma_start(out=bv_sb, in_=bv)
            nc.sync.dma_start(out=ones_sb, in_=onesd)
            # ones columns of v (col 64 of each head block)
            v_ones_ap = v_sb[:].rearrange("p s (h x) -> p s h x", x=VW)[:, :, :, DK:DK + 1]
            nc.sync.dma_start(
                out=v_ones_ap,
                in_=vones.rearrange("p (s h o) -> p s h o", s=NST, h=4))

            # ---- stage A: projections ----
            with tc.tile_pool(name="psA", bufs=1, space="PSUM") as psA:
                for (xT, w_sb, b_sb, out_sb) in (
                    (xqT, wq_sb, bq_sb, qT_sb),
                    (xkT, wk_sb, bk_sb, kT_sb),
                ):
                    for n4 in range(NQ4):
                        ppm = [psA.tile([P, 512], F32, tag=f"pp{m}", bufs=2,
                                        name=f"pp{m}")
                               for m in range(NM)]
                        for k in range(NK):
                            xt = xq_pool.tile([P, 512], F32R, tag="xt")
                            nc.sync.dma_start(
                                out=xt,
                                in_=xT[k * P:(k + 1) * P, n4 * 512:(n4 + 1) * 512])
                            for m in range(NM):
                                nc.tensor.matmul(
                                    ppm[m][:], w_sb[:, k, m * P:(m + 1) * P], xt[:],
                                    start=(k == 0), stop=(k == NK - 1))
                        for m in range(NM):
                            nc.vector.tensor_scalar_add(
                                out_sb[:, m, n4 * 512:(n4 + 1) * 512], ppm[m][:],
                                b_sb[:, m:m + 1])

                for st in range(NST):
                    pv = psA.tile([P, DH], F32, tag="pv", bufs=2)
                    for k in range(NK):
                        xvt = xv_pool.tile([P, P], F32R, tag="xvt")
                        nc.sync.dma_start(
                            out=xvt,
                            in_=xvT[k * P:(k + 1) * P, st * P:(st + 1) * P])
                        nc.tensor.matmul(pv[:], xvt[:], wv_sb[:, k, :],
                                         start=(k == 0), stop=False)
                    nc.tensor.matmul(pv[:], ones_sb[:], bv_sb[:],
                                     start=False, stop=True)
                    # strided copy into per-head 65-wide blocks (col 64 = ones)
                    v_dst = v_sb[:, st, :].rearrange(
                        "p (h x) -> p h x", x=VW)[:, :, 0:DK]
                    v_src = pv[:].rearrange("p (h x) -> p h x", x=DK)
                    nc.vector.tensor_copy(v_dst, v_src)

            # ---- stage B: attention ----
            with tc.tile_pool(name="psB", bufs=1, space="PSUM") as psB:
                for c in range(2):  # head pairs (2c, 2c+1)
                    for q4 in range(NQ4):
                        otp = [psB.tile([VW, 512], F32, tag=f"ot{i}", bufs=2,
                                        name=f"ot{i}")
                               for i in range(2)]
                        for kt in range(NST):
                            stp = psB.tile([P, 1024], F32, tag="stp", bufs=2)
                            for i in range(2):  # head within pair
                                pa = 64 * i
                                nc.tensor.matmul(
                                    stp[:, i * 512:(i + 1) * 512],
                                    kT_sb[pa:pa + DK, c, kt * P:(kt + 1) * P],
                                    qT_sb[pa:pa + DK, c, q4 * 512:(q4 + 1) * 512],
                                    start=True, stop=True)
                            et = e_pool.tile([P, 1024], F32R, tag="et")
                            nc.scalar.activation(
                                et[:], stp[:], mybir.ActivationFunctionType.Exp,
                                scale=0.125)
                            for i in range(2):
                                h = 2 * c + i
                                nc.tensor.matmul(
                                    otp[i][:], v_sb[:, kt, h * VW:(h + 1) * VW],
                                    et[:, i * 512:(i + 1) * 512],
                                    start=(kt == 0), stop=(kt == NST - 1),
                                    skip_group_check=True)
                        for i in range(2):
                            # normalize: rows 0-63 /= row 64
                            rs = nrm_pool.tile([P, 512], F32, tag="rs")
                            nc.vector.reciprocal(rs[DK:DK + 1, :],
                                                 otp[i][DK:DK + 1, :])
                            rs0 = nrm_pool.tile([1, 512], F32, tag="rs0")
                            nc.sync.dma_start(out=rs0, in_=rs[DK:DK + 1, :])
                            rb = nrm_pool.tile([DK, 512], F32, tag="rb")
                            nc.gpsimd.partition_broadcast(rb[:], rs0[:])
                            qs = slice(q4 * 512, (q4 + 1) * 512)
                            if i == 0:
                                nc.vector.tensor_mul(otn_sb[0:DK, c, qs],
                                                     otp[i][0:DK, :], rb[:])
                            else:
                                tmp = nrm_pool.tile([DK, 512], F32R, tag="tmp")
                                nc.vector.tensor_mul(tmp[:], otp[i][0:DK, :],
                                                     rb[:])
                                nc.sync.dma_start(out=otn_sb[DK:P, c, qs],
                                                  in_=tmp[:])

            # ---- stage C: output projection ----
            with tc.tile_pool(name="psC", bufs=1, space="PSUM") as psC:
                for t in range(NST):
                    yt = y_pool.tile([P, D], F32, tag="yt")
                    for n in range(2):
                        yps = psC.tile([P, 512], F32, tag="yps", bufs=4)
                        for c in range(NM):
                            nc.tensor.matmul(
                                yps[:], otn_sb[:, c, t * P:(t + 1) * P],
                                wo_sb[:, c, n * 512:(n + 1) * 512],
                                start=(c == 0), stop=(c == NM - 1))
                        nc.vector.tensor_copy(yt[:, n * 512:(n + 1) * 512],
                                              yps[:])
                    nc.sync.dma_start(out=yp[t * P:(t + 1) * P, :], in_=yt[:])

    nc.compile()
    return nc


_NC = None


def _get_program():
    global _NC
    if _NC is None:
        _NC = _build_program()
    return _NC


def _make_in_maps(Q, K, V, Wq, bq, Wk, bk, Wv, bv, Wo):
    qT = [np.ascontiguousarray(Q[b].T) for b in range(2)]
    kT = [np.ascontiguousarray(K[b].T) for b in range(2)]
    vT = [np.ascontiguousarray(V[b].T) for b in range(2)]
    in_maps = []
    for c in range(NCORES):
        b = c // 4
        g = c % 4
        cols = slice(g * DH, (g + 1) * DH)
        in_maps.append({
            "xqT": qT[b],
            "xkT": kT[b],
            "xvT": vT[b],
            "wq": np.ascontiguousarray(Wq[:, cols]),
            "wk": np.ascontiguousarray(Wk[:, cols]),
            "wv": np.ascontiguousarray(Wv[:, cols]),
            "wo": np.ascontiguousarray(Wo[cols, :]),
            "bq": np.ascontiguousarray(bq[cols].reshape(DH, 1)),
            "bk": np.ascontiguousarray(bk[cols].reshape(DH, 1)),
            "bv": np.ascontiguousarray(bv[cols].reshape(1, DH)),
            "onesd": np.ones((1, P), np.float32),
            "vones": np.ones((P, NST * 4), np.float32),
        })
    return in_maps


def run(inputs, trace=False):
    """Returns (full_output [2, S, D] float32, exec_time_ns or None)."""
    nc = _get_program()
    in_maps = _make_in_maps(
        np.asarray(inputs["Q"], np.float32), np.asarray(inputs["K"], np.float32),
        np.asarray(inputs["V"], np.float32), np.asarray(inputs["Wq"], np.float32),
        np.asarray(inputs["bq"], np.float32), np.asarray(inputs["Wk"], np.float32),
        np.asarray(inputs["bk"], np.float32), np.asarray(inputs["Wv"], np.float32),
        np.asarray(inputs["bv"], np.float32), np.asarray(inputs["Wo"], np.float32))
    res = run_bass_kernel_spmd(nc, in_maps, core_ids=list(range(NCORES)),
                               trace=trace)
    bo = np.asarray(inputs["bo"], np.float32)
    out = np.empty((2, S, D), np.float32)
    for b in range(2):
        acc = res.results[4 * b]["yp"].astype(np.float32).copy()
        for g in range(1, 4):
            acc += res.results[4 * b + g]["yp"]
        out[b] = acc + bo
    return out, res.exec_time_ns


def kernel(**inputs):
    out, _ = run(inputs, trace=False)
    return out
